# revision 1
# baseline (speedup 1.0000x reference)
"""Trainium2 Bass kernel for a single causal-attention transformer block.

Reference computation (per batch element b):
    xn  = rms_norm(x[b]) * rms_w
    q/k/v = xn @ Wq/Wk/Wv            (16 heads x 128 head dim)
    att = causal_softmax(q k^T / sqrt(2048)) @ v
    out[b] = att @ Wo + x[b]

Sharding (8 NeuronCores): tensor-parallel over heads x data-parallel over
batch.  Core c handles batch b = c // 4 and head-group i = c % 4 (4 heads,
512 columns of Wq/Wk/Wv, 512 rows of Wo).  Each core computes a partial
output  att_i @ Wo_i  for its batch element; the host sums the 4 partials
per batch and adds the residual.

On-device dataflow (per core):
  A. load x^T (bf16), col-sums of squares via ones-matmul -> rstd,
     broadcast rstd via ones-matmul, scale -> xn^T (bf16, resident).
  B. projections: qT/kT [dh, S] per head and v [S, dh] (bf16), fp32 PSUM.
  C. per (head, 512-query-chunk): scoresT tiles [t,s] = kT_tile^T @ qT_chunk,
     exp on ACT (no max-subtract needed: |scores| < ~1), causal mask via
     affine_select on diagonal tiles (with column truncation of the
     fully-masked region), pv-matmul accumulates attn^T [dh, s] in PSUM,
     an M=128 all-ones matmul accumulates the softmax denominator
     pre-broadcast across partitions, then a full-lane approx-reciprocal
     and one multiply normalize during evacuation.
  D. o_proj interleaved per query-chunk: once all heads finish chunk sc,
     out[s,:] += attn_i^T slices @ Wo_i for that chunk's s-tiles, with
     chunked fp32 output DMAs -- so the final matmuls and stores overlap
     the remaining attention work.

All matmul inputs are bf16 (fp32 PSUM accumulation).  rstd from the
RMS-norm is folded into the PSUM evacuation of q/k/v rather than scaling
x^T, and a single shared 8-bank PSUM pool lets the scheduler interleave
the RMS statistics with early projection matmuls.  Measured end-to-end
error vs the fp32 reference is ~9e-4 of the output absmax; measured HW
time ~376 us across 8 cores (slowest core).
"""

import numpy as np
import ml_dtypes

S = 2048          # sequence length
HID = 2048        # hidden dim
NH = 16           # total heads
DH = 128          # head dim
TP = 4            # head-group shards
DP = 2            # batch shards
KSH = HID // TP   # per-core key-dim shard (512)
NHS = KSH // DH   # heads per core (4)
NT = S // 128     # 128-row tiles along s/t/h (16)
NSC = S // 512    # 512-wide chunks along s (4)
EPS = 1e-5
TRUNC = True  # causal truncation of diagonal tiles

BF16 = None  # set lazily (concourse import)
_STATE = {}


def _build_nc():
    from contextlib import ExitStack

    import concourse.bacc as bacc
    import concourse.tile as tile
    from concourse import mybir

    F32 = mybir.dt.float32
    BF = mybir.dt.bfloat16
    AF = mybir.ActivationFunctionType

    nc = bacc.Bacc("TRN2")
    xt = nc.dram_tensor("xt", [HID, S], BF, kind="ExternalInput")
    wq = nc.dram_tensor("wq", [HID, KSH], BF, kind="ExternalInput")
    wk = nc.dram_tensor("wk", [HID, KSH], BF, kind="ExternalInput")
    wv = nc.dram_tensor("wv", [HID, KSH], BF, kind="ExternalInput")
    wo = nc.dram_tensor("wo", [KSH, HID], BF, kind="ExternalInput")
    out = nc.dram_tensor("out", [S, HID], F32, kind="ExternalOutput")

    with tile.TileContext(nc) as tc, ExitStack() as ctx:
        misc = ctx.enter_context(tc.tile_pool(name="misc", bufs=1))
        qt_pool = ctx.enter_context(tc.tile_pool(name="qt", bufs=NHS))
        kt_pool = ctx.enter_context(tc.tile_pool(name="kt", bufs=NHS))
        v_pool = ctx.enter_context(tc.tile_pool(name="v", bufs=NT))
        attn_pool = ctx.enter_context(tc.tile_pool(name="attn", bufs=NHS))
        probs_pool = ctx.enter_context(tc.tile_pool(name="probs", bufs=12))
        denb_pool = ctx.enter_context(tc.tile_pool(name="denb", bufs=4))

        ones_mat_bf = misc.tile([128, 128], BF, tag="ones_mat_bf", name="ones_mat_bf")
        nc.vector.memset(ones_mat_bf, 1.0)
        eps_sb = misc.tile([128, 1], F32, tag="eps_sb", name="eps_sb")
        nc.vector.memset(eps_sb, EPS)
        rstd_b = misc.tile([128, S], F32, tag="rstd_b", name="rstd_b")
        # rstd transposed to per-partition layout: rstd_colT[p, st] = rstd[st*128+p]
        rstd_colT = misc.tile([128, NT], F32, tag="rstd_colT", name="rstd_colT")
        ident = misc.tile([128, 128], F32, tag="ident", name="ident")
        nc.vector.memset(ident, 1.0)
        nc.gpsimd.affine_select(
            out=ident, in_=ident, compare_op=mybir.AluOpType.is_equal,
            fill=0.0, base=0, channel_multiplier=1, pattern=[[-1, 128]],
        )

        # ---------------- phases A+B (xnT + projections) ----------------
        with ExitStack() as ab:
            xnt_pool = ab.enter_context(
                tc.tile_pool(name="xnt", bufs=NT, side="right")
            )
            xsq_pool = ab.enter_context(
                tc.tile_pool(name="xsq", bufs=3, side="right")
            )
            w_pool = ab.enter_context(
                tc.tile_pool(name="wstream", bufs=NT + 4, side="right")
            )

            # single shared PSUM pool (8 banks): lets the scheduler interleave
            # the ss accumulation with early projection matmuls instead of
            # serializing phase A before phase B
            pp = ctx.enter_context(tc.tile_pool(name="pp", bufs=8, space="PSUM"))

            xnt = []
            ss = [pp.tile([128, 512], F32, tag="pp", name="ss") for _ in range(NSC)]
            for ht in range(NT):
                t = xnt_pool.tile([128, S], BF, tag="xnt", name="xnt")
                # split-row DMAs pipeline the squares behind the loads; the
                # first tile lands in quarters (parallel queues) so the first
                # matmul issues sooner
                nparts = 4 if ht == 0 else 2
                step = S // nparts
                for p_ in range(nparts):
                    nc.sync.dma_start(
                        out=t[:, p_ * step:(p_ + 1) * step],
                        in_=xt[ht * 128:(ht + 1) * 128, p_ * step:(p_ + 1) * step],
                    )
                xnt.append(t)
                sq = xsq_pool.tile([128, S], BF, tag="xsq", name="xsq")
                nc.vector.tensor_mul(sq[:, 0:S // 2], t[:, 0:S // 2], t[:, 0:S // 2])
                nc.vector.tensor_mul(sq[:, S // 2:], t[:, S // 2:], t[:, S // 2:])
                # M=128 all-ones stationary: every partition gets the column
                # sum, so rstd lands pre-broadcast
                for sc in range(NSC):
                    nc.tensor.matmul(
                        ss[sc],
                        ones_mat_bf,
                        sq[:, sc * 512:(sc + 1) * 512],
                        start=(ht == 0),
                        stop=(ht == NT - 1),
                    )
            for sc in range(NSC):
                cs = slice(sc * 512, (sc + 1) * 512)
                # sqrt(mean + eps), then reciprocal -> rstd (all lanes)
                mtmp = denb_pool.tile([128, 512], F32, tag="denb", name="mtmp")
                nc.scalar.activation(
                    mtmp, ss[sc], AF.Sqrt, bias=eps_sb, scale=1.0 / HID
                )
                nc.vector.reciprocal_approx_fast(rstd_b[:, cs], mtmp)

            # PE-transpose rstd_b slices to get per-partition rstd columns
            for st in range(NT):
                ptr = pp.tile([128, 512], F32, tag="pp", name="pp")
                nc.tensor.transpose(
                    ptr[:, 0:128], rstd_b[:, st * 128:(st + 1) * 128], ident
                )
                nc.vector.tensor_copy(rstd_colT[:, st:st + 1], ptr[:, 0:1])

            # --- q/k projections: qT/kT [dh, S] per head, stationary = W tile
            qts, kts = [], []
            for (w_dram, dst_list) in ((wq, qts), (wk, kts)):
                wts = []
                for ht in range(NT):
                    wt = w_pool.tile([128, KSH], BF, tag="w", name="w")
                    nc.sync.dma_start(
                        out=wt, in_=w_dram[ht * 128:(ht + 1) * 128, :]
                    )
                    wts.append(wt)
                for dt in range(NHS):
                    dst = (qt_pool if dst_list is qts else kt_pool).tile(
                        [128, S], BF, tag="qt", name="qt" if dst_list is qts else "kt"
                    )
                    ps = [pp.tile([128, 512], F32, tag="pp", name="pp") for _ in range(NSC)]
                    for ht in range(NT):
                        lhsT = wts[ht][:, dt * 128:(dt + 1) * 128]
                        for sc in range(NSC):
                            nc.tensor.matmul(
                                ps[sc],
                                lhsT,
                                xnt[ht][:, sc * 512:(sc + 1) * 512],
                                start=(ht == 0),
                                stop=(ht == NT - 1),
                            )
                    for sc in range(NSC):
                        cs = slice(sc * 512, (sc + 1) * 512)
                        # fold rstd[s] (free axis here) into the evacuation
                        nc.vector.tensor_mul(dst[:, cs], ps[sc], rstd_b[:, cs])
                    dst_list.append(dst)

            # --- v projection: natural layout [S, 512], stationary = xnT slice
            wvts = []
            for ht in range(NT):
                wt = w_pool.tile([128, KSH], BF, tag="w", name="w")
                nc.sync.dma_start(out=wt, in_=wv[ht * 128:(ht + 1) * 128, :])
                wvts.append(wt)
            v_sb = []
            for st in range(NT):
                psv = pp.tile([128, 512], F32, tag="pp", name="pp")
                for ht in range(NT):
                    nc.tensor.matmul(
                        psv,
                        xnt[ht][:, st * 128:(st + 1) * 128],
                        wvts[ht],
                        start=(ht == 0),
                        stop=(ht == NT - 1),
                    )
                vt = v_pool.tile([128, KSH], BF, tag="v", name="v")
                # fold rstd[s] (partition axis here) into the evacuation
                nc.vector.tensor_scalar_mul(vt, psv, rstd_colT[:, st:st + 1])
                v_sb.append(vt)
        # xnt/xsq/wstream released here

        wo_pool = ctx.enter_context(tc.tile_pool(name="wo", bufs=NHS, side="right"))
        out_pool = ctx.enter_context(tc.tile_pool(name="outp", bufs=4, side="right"))
        wo_sb = []
        for c in range(NHS):
            wt = wo_pool.tile([128, HID], BF, tag="wo", name="wo")
            nc.sync.dma_start(out=wt, in_=wo[c * 128:(c + 1) * 128, :])
            wo_sb.append(wt)

        # -------- phases C+D: attention, with o_proj interleaved per chunk ---
        # sc-outer / head-inner: once all 4 heads finish query-chunk sc, the
        # o_proj for that chunk's four s-tiles runs immediately, so the final
        # matmul phase and output DMAs overlap the remaining attention work.
        # Denominators: M=128 all-ones stationary -> every PSUM partition gets
        # the column sum (same N-cycle cost as M=1, and the result is already
        # broadcast, so the reciprocal runs full-lane straight off PSUM).
        attn_sb = [
            attn_pool.tile([128, S], BF, tag="attn", name="attn")
            for _ in range(NHS)
        ]
        for sc in range(NSC):
            cs = slice(sc * 512, (sc + 1) * 512)
            ntt = 4 * (sc + 1)
            for hd in range(NHS):
                at = attn_sb[hd]
                hs = slice(hd * 128, (hd + 1) * 128)
                ps_at = pp.tile([128, 512], F32, tag="pp", name="at")
                ps_dn = pp.tile([128, 512], F32, tag="pp", name="dn")
                for tt in range(ntt):
                    # diagonal tiles: columns below 128*j are fully masked --
                    # skip them in the matmuls / exp / mask (causal truncation)
                    j = tt - 4 * sc
                    c0 = 128 * j if (j > 0 and TRUNC) else 0
                    ps_s = pp.tile([128, 512], F32, tag="pp", name="pp")
                    nc.tensor.matmul(
                        ps_s[:, c0:],
                        kts[hd][:, tt * 128:(tt + 1) * 128],
                        qts[hd][:, sc * 512 + c0:(sc + 1) * 512],
                        start=True,
                        stop=True,
                    )
                    pt = probs_pool.tile([128, 512], BF, tag="probs", name="probs")
                    nc.scalar.activation(pt[:, c0:], ps_s[:, c0:], AF.Exp)
                    if j >= 0:
                        # keep where (f + c0) - t - 128*j >= 0 within the window
                        nc.gpsimd.affine_select(
                            out=pt[:, c0:],
                            in_=pt[:, c0:],
                            compare_op=mybir.AluOpType.is_ge,
                            fill=0.0,
                            base=c0 - 128 * j,
                            channel_multiplier=-1,
                            pattern=[[1, 512 - c0]],
                        )
                    nc.tensor.matmul(
                        ps_at[:, c0:],
                        v_sb[tt][:, hs],
                        pt[:, c0:],
                        start=(tt == 0),
                        stop=(tt == ntt - 1),
                    )
                    nc.tensor.matmul(
                        ps_dn[:, c0:],
                        ones_mat_bf,
                        pt[:, c0:],
                        start=(tt == 0),
                        stop=(tt == ntt - 1),
                    )
                denb = denb_pool.tile([128, 512], F32, tag="denb", name="denb")
                nc.vector.reciprocal_approx_fast(denb, ps_dn)
                nc.vector.tensor_mul(at[:, cs], ps_at, denb)

            # o_proj for the four s-tiles of this chunk (all heads now done)
            for st in range(4 * sc, 4 * sc + 4):
                ot = out_pool.tile([128, HID], F32, tag="outp", name="outp")
                ps_o = [
                    pp.tile([128, 512], F32, tag="pp", name="po")
                    for _ in range(NSC)
                ]
                for c in range(NHS):
                    lhsT = attn_sb[c][:, st * 128:(st + 1) * 128]
                    for ec in range(NSC):
                        nc.tensor.matmul(
                            ps_o[ec],
                            lhsT,
                            wo_sb[c][:, ec * 512:(ec + 1) * 512],
                            start=(c == 0),
                            stop=(c == NHS - 1),
                        )
                for ec in range(NSC):
                    es = slice(ec * 512, (ec + 1) * 512)
                    nc.vector.tensor_copy(ot[:, es], ps_o[ec])
                    nc.sync.dma_start(
                        out=out[st * 128:(st + 1) * 128, es], in_=ot[:, es]
                    )

    return nc


def get_nc():
    if "nc" not in _STATE:
        nc = _build_nc()
        nc.finalize()
        _STATE["nc"] = nc
    return _STATE["nc"]


def make_in_maps(x, rms_w, Wq, Wk, Wv, Wo):
    """Host-side sharding: returns one input dict per core (8 cores)."""
    bf16 = ml_dtypes.bfloat16
    scale = 1.0 / np.sqrt(np.float32(HID))
    rw = rms_w.astype(np.float32)[:, None]
    wq_f = (rw * Wq.astype(np.float32) * scale)
    wk_f = (rw * Wk.astype(np.float32))
    wv_f = (rw * Wv.astype(np.float32))
    in_maps = []
    for c in range(DP * TP):
        b, i = divmod(c, TP)
        cols = slice(i * KSH, (i + 1) * KSH)
        in_maps.append({
            "xt": np.ascontiguousarray(x[b].T).astype(bf16),
            "wq": np.ascontiguousarray(wq_f[:, cols]).astype(bf16),
            "wk": np.ascontiguousarray(wk_f[:, cols]).astype(bf16),
            "wv": np.ascontiguousarray(wv_f[:, cols]).astype(bf16),
            "wo": np.ascontiguousarray(Wo.astype(np.float32)[cols, :]).astype(bf16),
        })
    return in_maps


def kernel(x, rms_w, Wq, Wk, Wv, Wo, _trace=False, _results_out=None):
    from concourse.bass_utils import run_bass_kernel_spmd

    nc = get_nc()
    in_maps = make_in_maps(x, rms_w, Wq, Wk, Wv, Wo)
    kw = {}
    if _trace:
        kw = dict(trace=True, trace_cores=list(range(DP * TP)))
    res = run_bass_kernel_spmd(
        nc, in_maps, core_ids=list(range(DP * TP)), **kw
    )
    if _results_out is not None:
        _results_out.append(res)
    out = np.empty((DP, S, HID), np.float32)
    for b in range(DP):
        acc = x[b].astype(np.float32).copy()
        for i in range(TP):
            acc += res.results[b * TP + i]["out"]
        out[b] = acc
    return out



# revision 14
# speedup vs baseline: 1.3779x; 1.3779x over previous
"""Trainium2 Bass kernel for a single causal-attention transformer block.

Reference computation (per batch element b):
    xn  = rms_norm(x[b]) * rms_w
    q/k/v = xn @ Wq/Wk/Wv            (16 heads x 128 head dim)
    att = causal_softmax(q k^T / sqrt(2048)) @ v
    out[b] = att @ Wo + x[b]

Sharding (8 NeuronCores): tensor-parallel over heads x data-parallel over
batch.  Core c handles batch b = c // 4 and head-group i = c % 4 (4 heads,
512 columns of Wq/Wk/Wv, 512 rows of Wo).  Each core computes a partial
output  att_i @ Wo_i  for its batch element; the host sums the 4 partials
per batch (scaled by 2^-10, see below) and adds the residual.

fp8 DoubleRow scheme: all heavy matmuls except the scores run as
float8e4 (e4m3) MatmulPerfMode.DoubleRow, which contracts 256 elements
per instruction (two 128-deep planes packed in an extra free dim of 2)
at ~0.43 ns/output-column -- 2x the bf16 rate, measured 154 TF/s.

  - x arrives pre-transposed/quantized from the host as pair tiles
    xp[128, j, i, s] = x^T[(2j+i)*128 + p, s] (fp8).
  - Wq/Wk/Wv/Wo arrive fp8, scaled by 2^5 to avoid e4m3 subnormals
    (|W| ~ 0.022 would quantize terribly at 2^-10 granularity).
  - RMS stats: squares on gpsimd (fp8), column sums via an all-ones
    (value 2^-5) DoubleRow matmul; sqrt/reciprocal produce
    rstd_b = rstd * 2^-5 broadcast on all partitions, so the q/k PSUM
    evacuation (x 2^5 Wq^T x) * rstd_b lands TRUE q/k in bf16.
  - scores stay bf16 (contraction is only dh=128, DoubleRow can't pair);
    the 1/sqrt(2048) score scale is applied inside the exp activation,
    which writes fp8 probs directly.
  - causal mask: affine_select on fp8 probs for diagonal tiles, widened
    to the pair window [c0p:] so the partner plane's dead columns are
    zeroed before the paired PV matmul streams them.
  - PV + softmax-denominator accumulate per query-chunk via DoubleRow
    over t-tile pairs (v in fp8 pair tiles, ones value 2^-5), so the
    full-lane reciprocal yields 2^5/den and the attn evacuation writes
    2^5*attn in fp8 (healthy range; raw attn ~ 1/sqrt(L) is subnormal).
  - o_proj: DoubleRow over the 4 dh-blocks (2 pairs); PSUM holds
    2^10 * partial, evacuated as bf16 and divided by 2^10 on the host.

Measured HW time ~128 us across 8 cores (slowest core) vs 377 us for the
all-bf16 version; end-to-end absmax error ~6e-3 of the output absmax
(fp8 quantization noise; threshold is 2e-2).
"""

import numpy as np
import ml_dtypes

S = 2048          # sequence length
HID = 2048        # hidden dim
NH = 16           # total heads
DH = 128          # head dim
TP = 4            # head-group shards
DP = 2            # batch shards
KSH = HID // TP   # per-core key-dim shard (512)
NHS = KSH // DH   # heads per core (4)
NT = S // 128     # 128-row tiles along s/t (16)
NSC = S // 512    # 512-wide chunks along s (4)
NJP = HID // 256  # hidden-dim pairs (8)
EPS = 1e-5
WS = 32.0         # fp8 weight pre-scale (2^5)
OSC = 1.0 / (WS * WS)  # host-side unscale of output partials

_STATE = {}


def _build_nc():
    from contextlib import ExitStack

    import concourse.bacc as bacc
    import concourse.tile as tile
    from concourse import mybir

    F32 = mybir.dt.float32
    BF = mybir.dt.bfloat16
    F8 = mybir.dt.float8e4
    AF = mybir.ActivationFunctionType
    PM = mybir.MatmulPerfMode
    DR = PM.DoubleRow
    SCALE = 1.0 / float(np.sqrt(np.float32(HID)))

    nc = bacc.Bacc("TRN2")
    xp = nc.dram_tensor("xp", [128, NJP, 2, S], F8, kind="ExternalInput")
    wq = nc.dram_tensor("wq", [128, NJP, 2, KSH], F8, kind="ExternalInput")
    wk = nc.dram_tensor("wk", [128, NJP, 2, KSH], F8, kind="ExternalInput")
    wv = nc.dram_tensor("wv", [128, NJP, 2, KSH], F8, kind="ExternalInput")
    wo = nc.dram_tensor("wo", [128, 2, 2, HID], F8, kind="ExternalInput")
    out = nc.dram_tensor("out", [S, HID], BF, kind="ExternalOutput")

    with tile.TileContext(nc) as tc, ExitStack() as ctx:
        misc = ctx.enter_context(tc.tile_pool(name="misc", bufs=1))
        qt_pool = ctx.enter_context(tc.tile_pool(name="qt", bufs=NHS))
        kt_pool = ctx.enter_context(tc.tile_pool(name="kt", bufs=NHS))
        v_pool = ctx.enter_context(tc.tile_pool(name="v", bufs=NJP))
        attn_pool = ctx.enter_context(tc.tile_pool(name="attn", bufs=2))
        probs_pool = ctx.enter_context(tc.tile_pool(name="probs", bufs=12))
        denb_pool = ctx.enter_context(tc.tile_pool(name="denb", bufs=4))
        wo_pool = ctx.enter_context(tc.tile_pool(name="wo", bufs=1, side="right"))
        out_pool = ctx.enter_context(tc.tile_pool(name="outp", bufs=4, side="right"))

        # all-ones (value 2^-5) DoubleRow stationary: used for the RMS
        # column sums and the softmax denominators
        ones8 = misc.tile([128, 2, 128], F8, tag="ones8", name="ones8")
        nc.vector.memset(ones8, 1.0 / WS)
        eps_sb = misc.tile([128, 1], F32, tag="eps_sb", name="eps_sb")
        nc.vector.memset(eps_sb, EPS * WS * WS)
        rstd_b = misc.tile([128, S], F32, tag="rstd_b", name="rstd_b")
        # rstd transposed to per-partition layout: rstd_colT[p, st] = rstd[st*128+p]
        rstd_colT = misc.tile([128, NT], F32, tag="rstd_colT", name="rstd_colT")
        ident = misc.tile([128, 128], F32, tag="ident", name="ident")
        nc.vector.memset(ident, 1.0)
        nc.gpsimd.affine_select(
            out=ident, in_=ident, compare_op=mybir.AluOpType.is_equal,
            fill=0.0, base=0, channel_multiplier=1, pattern=[[-1, 128]],
        )

        pp = ctx.enter_context(tc.tile_pool(name="pp", bufs=8, space="PSUM"))

        # ---------------- phases A+B (RMS stats + projections) ----------------
        with ExitStack() as ab:
            xp_pool = ab.enter_context(tc.tile_pool(name="xp", bufs=NJP, side="right"))
            sq_pool = ab.enter_context(tc.tile_pool(name="sq", bufs=NJP, side="right"))
            w_pool = ab.enter_context(tc.tile_pool(name="wstream", bufs=3, side="right"))

            xpt = []
            for j in range(NJP):
                t = xp_pool.tile([128, 2, S], F8, tag="xp", name="xp")
                # split halves -> parallel DMA queues
                nc.sync.dma_start(out=t[:, 0, :], in_=xp[:, j, 0, :])
                nc.sync.dma_start(out=t[:, 1, :], in_=xp[:, j, 1, :])
                xpt.append(t)
            w8 = {}
            for name, dram in (("wq", wq), ("wk", wk), ("wv", wv)):
                wt = w_pool.tile([128, NJP, 2, KSH], F8, tag="w", name=name)
                nc.sync.dma_start(out=wt, in_=dram[:, :, :, :])
                w8[name] = wt
            wot = wo_pool.tile([128, 2, 2, HID], F8, tag="wo", name="wo")
            nc.sync.dma_start(out=wot, in_=wo[:, :, :, :])

            # squares (gpsimd, fp8) -> DoubleRow ones-matmul column sums.
            # ss chunks hold sum(x^2) * 2^-5 pre-broadcast on all partitions.
            # NOTE on PSUM chains everywhere below: a PSUM bank supports only
            # ONE open accumulation group at a time on hardware -- opening a
            # second start/stop chain in the same bank while the first is
            # still open silently corrupts it.  All chains sharing a
            # [128,512] tile therefore run sequentially (h-outer loops).
            ss = [pp.tile([128, 512], F32, tag="pp", name="ss") for _ in range(4)]
            sqs = []
            for j in range(NJP):
                sq = sq_pool.tile([128, 2, S], F8, tag="sq", name="sq")
                # squares split round-robin across DVE/ACT/gpsimd so the
                # serialized-engine latency doesn't gate rstd
                eng = j % 3
                if eng == 0:
                    nc.vector.tensor_mul(sq, xpt[j], xpt[j])
                elif eng == 1:
                    nc.scalar.activation(sq, xpt[j], AF.Square)
                else:
                    nc.gpsimd.tensor_mul(sq, xpt[j], xpt[j])
                sqs.append(sq)
            for h in range(2):
                for j in range(NJP):
                    for c in range(4):
                        ch = 2 * c + h
                        nc.tensor.matmul(
                            ss[c][:, h * 256:h * 256 + 256],
                            ones8,
                            sqs[j][:, :, ch * 256:(ch + 1) * 256],
                            start=(j == 0),
                            stop=(j == NJP - 1),
                            perf_mode=DR,
                        )
            for c in range(4):
                cs = slice(c * 512, (c + 1) * 512)
                # mtmp = 2^5 * sqrt(ms + eps); reciprocal -> rstd * 2^-5
                mtmp = denb_pool.tile([128, 512], F32, tag="denb", name="mtmp")
                nc.scalar.activation(
                    mtmp, ss[c], AF.Sqrt, bias=eps_sb, scale=WS * WS * WS / HID
                )
                nc.vector.reciprocal_approx_fast(rstd_b[:, cs], mtmp)

            # --- q/k projections: qT/kT [dh, S] per head (bf16, true scale)
            qts, kts = [], []
            for wname, pool, dst_list in (("wq", qt_pool, qts), ("wk", kt_pool, kts)):
                w8t = w8[wname]
                for dt in range(NHS):
                    ps4 = [pp.tile([128, 512], F32, tag="pp", name="pq")
                           for _ in range(4)]
                    for h in range(2):
                        for c in range(4):
                            ch = 2 * c + h
                            for j in range(NJP):
                                nc.tensor.matmul(
                                    ps4[c][:, h * 256:h * 256 + 256],
                                    w8t[:, j, :, dt * 128:(dt + 1) * 128],
                                    xpt[j][:, :, ch * 256:(ch + 1) * 256],
                                    start=(j == 0),
                                    stop=(j == NJP - 1),
                                    perf_mode=DR,
                                )
                    dst = pool.tile([128, S], BF, tag="qt", name=wname + "t")
                    for c in range(4):
                        cs = slice(c * 512, (c + 1) * 512)
                        nc.vector.tensor_mul(dst[:, cs], ps4[c], rstd_b[:, cs])
                    dst_list.append(dst)

            # PE-transpose rstd_b slices to get per-partition rstd columns
            # (emitted after q/k so these PSUM allocations, which wait on
            # rstd, don't block the projection chains in the pool queue)
            for st in range(NT):
                ptr = pp.tile([128, 512], F32, tag="pp", name="ptr")
                nc.tensor.transpose(
                    ptr[:, 0:128], rstd_b[:, st * 128:(st + 1) * 128], ident
                )
                nc.vector.tensor_copy(rstd_colT[:, st:st + 1], ptr[:, 0:1])

            # --- v projection: pair tiles vp[tp][p, i, m] = v[(2tp+i)*128+p, m]
            w8v = w8["wv"]
            vp = [v_pool.tile([128, 2, KSH], F8, tag="v", name="v")
                  for _ in range(NJP)]
            for st in range(NT):
                psv = pp.tile([128, 512], F32, tag="pp", name="pv")
                for h in range(2):
                    for j in range(NJP):
                        nc.tensor.matmul(
                            psv[:, h * 256:(h + 1) * 256],
                            xpt[j][:, :, st * 128:(st + 1) * 128],
                            w8v[:, j, :, h * 256:(h + 1) * 256],
                            start=(j == 0),
                            stop=(j == NJP - 1),
                            perf_mode=DR,
                        )
                nc.vector.tensor_scalar_mul(
                    vp[st // 2][:, st % 2, :], psv, rstd_colT[:, st:st + 1]
                )
        # xp/sq/wstream released here

        # -------- phases C+D: attention, with o_proj interleaved per chunk ---
        attnp = [attn_pool.tile([128, 2, S], F8, tag="attn", name="attn")
                 for _ in range(2)]
        for sc in range(NSC):
            cs = slice(sc * 512, (sc + 1) * 512)
            npair = 2 * (sc + 1)
            for hd in range(NHS):
                ps_at = pp.tile([128, 512], F32, tag="pp", name="at")
                ps_dn = pp.tile([128, 512], F32, tag="pp", name="dn")
                # pass 1: probs for every t-tile pair of this (sc, hd)
                ptps = []
                for jp in range(npair):
                    # pair truncation: columns below c0p are fully masked in
                    # BOTH planes of this t-tile pair
                    c0p = max(0, 128 * (2 * jp - 4 * sc))
                    ptp = probs_pool.tile([128, 2, 512], F8, tag="probs",
                                          name="probs")
                    ptps.append(ptp)
                    for ii in range(2):
                        tt = 2 * jp + ii
                        jd = tt - 4 * sc
                        c0 = 128 * jd if jd > 0 else 0
                        ps_s = pp.tile([128, 512], F32, tag="pp", name="ps")
                        nc.tensor.matmul(
                            ps_s[:, c0:],
                            kts[hd][:, tt * 128:(tt + 1) * 128],
                            qts[hd][:, sc * 512 + c0:(sc + 1) * 512],
                            start=True,
                            stop=True,
                        )
                        nc.scalar.activation(
                            ptp[:, ii, c0:], ps_s[:, c0:], AF.Exp, scale=SCALE
                        )
                        if jd >= 0:
                            # keep where col >= t + 128*jd
                            nc.gpsimd.affine_select(
                                out=ptp[:, ii, c0:],
                                in_=ptp[:, ii, c0:],
                                compare_op=mybir.AluOpType.is_ge,
                                fill=0.0,
                                base=c0 - 128 * jd,
                                channel_multiplier=-1,
                                pattern=[[1, 512 - c0]],
                            )
                        if c0p < c0:
                            # partner plane of the pair streams [c0p:c0) too;
                            # zero this never-written strip
                            nc.gpsimd.memset(ptp[:, ii, c0p:c0], 0.0)
                # pass 2: PV + denominator, one open chain per bank at a time
                for qc in range(2):
                    hi = (qc + 1) * 256
                    jlist = [jp for jp in range(npair)
                             if max(0, 128 * (2 * jp - 4 * sc)) < hi]
                    for idx, jp in enumerate(jlist):
                        c0p = max(0, 128 * (2 * jp - 4 * sc))
                        lo = max(qc * 256, c0p)
                        st_, sp_ = (idx == 0), (idx == len(jlist) - 1)
                        nc.tensor.matmul(
                            ps_at[:, lo:hi],
                            vp[jp][:, :, hd * 128:(hd + 1) * 128],
                            ptps[jp][:, :, lo:hi],
                            start=st_, stop=sp_, perf_mode=DR,
                        )
                        nc.tensor.matmul(
                            ps_dn[:, lo:hi],
                            ones8,
                            ptps[jp][:, :, lo:hi],
                            start=st_, stop=sp_, perf_mode=DR,
                        )
                denb = denb_pool.tile([128, 512], F32, tag="denb", name="denb")
                nc.vector.reciprocal_approx_fast(denb, ps_dn)
                # 2^5 * attn (ps_dn carries 2^-5); healthy fp8 range
                nc.vector.tensor_mul(attnp[hd // 2][:, hd % 2, cs], ps_at, denb)

            # o_proj for the four s-tiles of this chunk (all heads now done)
            for st in range(4 * sc, 4 * sc + 4):
                ps_o = [pp.tile([128, 512], F32, tag="pp", name="po")
                        for _ in range(4)]
                for h in range(2):
                    for c in range(4):
                        ch = 2 * c + h
                        for jp in range(2):
                            nc.tensor.matmul(
                                ps_o[c][:, h * 256:h * 256 + 256],
                                attnp[jp][:, :, st * 128:(st + 1) * 128],
                                wot[:, jp, :, ch * 256:(ch + 1) * 256],
                                start=(jp == 0),
                                stop=(jp == 1),
                                perf_mode=DR,
                            )
                ot = out_pool.tile([128, HID], BF, tag="outp", name="outp")
                for ec in range(4):
                    es = slice(ec * 512, (ec + 1) * 512)
                    nc.vector.tensor_copy(ot[:, es], ps_o[ec])
                    nc.sync.dma_start(
                        out=out[st * 128:(st + 1) * 128, es], in_=ot[:, es]
                    )

    return nc


def get_nc():
    if "nc" not in _STATE:
        nc = _build_nc()
        nc.finalize()
        _STATE["nc"] = nc
    return _STATE["nc"]


def _pair_pack(a):
    """[256*n, m] -> [128, n, 2, m] with [p, j, i, m] = a[(2j+i)*128 + p, m]."""
    n = a.shape[0] // 256
    return np.ascontiguousarray(
        a.reshape(n, 2, 128, a.shape[1]).transpose(2, 0, 1, 3)
    )


def make_in_maps(x, rms_w, Wq, Wk, Wv, Wo):
    """Host-side sharding: returns one input dict per core (8 cores)."""
    e4 = ml_dtypes.float8_e4m3
    rw = rms_w.astype(np.float32)[:, None]
    wq_f = rw * Wq.astype(np.float32) * WS
    wk_f = rw * Wk.astype(np.float32) * WS
    wv_f = rw * Wv.astype(np.float32) * WS
    wo_f = Wo.astype(np.float32) * WS
    xp_b = [
        _pair_pack(np.ascontiguousarray(x[b].astype(np.float32).T)).astype(e4)
        for b in range(DP)
    ]
    in_maps = []
    for c in range(DP * TP):
        b, i = divmod(c, TP)
        cols = slice(i * KSH, (i + 1) * KSH)
        in_maps.append({
            "xp": xp_b[b],
            "wq": _pair_pack(wq_f[:, cols]).astype(e4),
            "wk": _pair_pack(wk_f[:, cols]).astype(e4),
            "wv": _pair_pack(wv_f[:, cols]).astype(e4),
            "wo": _pair_pack(wo_f[cols, :]).astype(e4),
        })
    return in_maps


def kernel(x, rms_w, Wq, Wk, Wv, Wo, _trace=False, _results_out=None):
    from concourse.bass_utils import run_bass_kernel_spmd

    nc = get_nc()
    in_maps = make_in_maps(x, rms_w, Wq, Wk, Wv, Wo)
    kw = {}
    if _trace:
        kw = dict(trace=True, trace_cores=list(range(DP * TP)))
    res = run_bass_kernel_spmd(
        nc, in_maps, core_ids=list(range(DP * TP)), **kw
    )
    if _results_out is not None:
        _results_out.append(res)
    out = np.empty((DP, S, HID), np.float32)
    for b in range(DP):
        acc = x[b].astype(np.float32).copy()
        for i in range(TP):
            acc += res.results[b * TP + i]["out"].astype(np.float32) * OSC
        out[b] = acc
    return out
